# revision 31
# baseline (speedup 1.0000x reference)
"""Multi-head attention + residual + LayerNorm, 8-core SPMD Trainium2 kernel.

v8 (~406us, from 462us bf16 baseline; rel err 1.06e-2 < 2e-2 budget):

fp8: all four projection GEMMs (Q/K/V/O) run fp8-e4m3 with DoubleRow perf
mode: operands are [128, 2, *] k-pair APs, so each matmul contracts 256
rows — half the matmuls of the bf16 chains at the same N=512.  Weights are
pre-scaled by WS=32 on the host (else ~40% of N(0,1/H) weights land in the
e4m3 subnormal range); the 1/WS rescale folds into the existing post-chain
tensor_scalar ops.  For the O projection the WS^2=1024 scale is absorbed by
pre-scaling the residual hs_q by 1024 (LayerNorm is scale-invariant) and
accumulating softmax sums at 1/WS scale (making cT = WS*ctx fit fp8 range).
Scores/ctx stay bf16: probs in e4m3 measured 8.6e-2 rel err (>> budget),
so the fp8 ctx DoubleRow variant is numerically off the table.

tail: pair 7 has no projection chains left to drain, so the O-projection's
first half (hp 0-5, j=0..1) accumulates into SBUF xp tiles (seeded with the
1024x residual) inside pair 7's drain slots; phase D only runs j=2..3 plus
LayerNorm.  The 4MB xp residual DMA is write-after-write gated on a kT[7]
token so the dependency-free loads can't race 250us ahead of the attention
loads on the sync DGE queue.

prologue: each dma_start costs ~650ns of serialized DGE issue time per
queue, so hsT loads as 8 full-row DMAs on sync while wq/wk load on the
(otherwise idle) gpsimd queue.

v2: restructured attention using PE array tiling for concurrency.
  - scores: head-pair row-tiled (64x128 mode) -> two K=64 matmuls run
    concurrently in the top/bottom halves of the PE array.
  - ctx: col-tiled 128x32 mode -> per qc, 4 concurrent M=32 matmuls
    (2 heads x d-lo/d-hi) into one PSUM bank; softmax sums via 4 more
    concurrent M=1 ones-matmuls into a dedicated sums bank (rows 0/32/64/96),
    replacing the old 65-column "ones trick".
  - phase A shrunk: only transposes + V st0/1 + K0/Q0 run before attention;
    remaining V/K/Q chains are emitted just-in-time inside the attention
    pair windows through a single shared PSUM matmul slot.
  - reciprocal_approx_fast instead of reciprocal (5x faster, ~18 bits).

Sharding: 8 shards = (batch b, sequence half sb).  Each core owns 1024 query
rows of one batch but computes K/V over the batch's full 2048 keys.

PSUM map during attention (8 banks):
  sc  [128,1024] f32 x2  banks 0-3   scores (qc pair-tile: head A | head B)
  cx  [128, 512] f32 x2  banks 4-5   ctx accumulators (A rows 0:64, B 64:128)
  sm  [128, 512] f32 x1  bank 6      sums rows 0/32/64/96 = (A,q0)(B,q0)(A,q1)(B,q1)
  mm  [128, 512] f32 x1  bank 7      shared proj-chain slot (K/Q/V)
"""

import numpy as np

import concourse.bass as bass
import concourse.mybir as mybir
import concourse.tile as tile
from concourse import bacc
from concourse.bass_utils import run_bass_kernel_spmd

F32 = mybir.dt.float32
BF16 = mybir.dt.bfloat16
F8 = mybir.dt.float8e4
AF = mybir.ActivationFunctionType
OP = mybir.AluOpType
DR = mybir.MatmulPerfMode.DoubleRow

B, S, H = 4, 2048, 1024
NH, HD = 16, 64
SH = S // 2          # own query rows per core
N_CORES = 8
EPS = 1e-12
WS = 32.0            # fp8 weight pre-scale (host multiplies W.T by WS)
RS = 1.0 / WS

HT = H // 128        # 8 contraction tiles
HT2 = HT // 2        # 4 double-row contraction tiles
ST = S // 128        # 16 key tiles
QB = SH // 512       # 2 q chunks
HP = NH // 2         # 8 head-pair tiles

_CACHED_NC = {}


def _emit(tc, ln_id, bo_zero=True, bv_zero=True):
    nc = tc.nc
    # hs_q is pre-scaled by 1024 on the host: the residual add x = attn + hs
    # runs at 1024x scale (absorbing the fp8 WS^2 weight scaling of the O
    # projection with zero extra ops) and LayerNorm is scale-invariant.
    hs_q = nc.dram_tensor("hs_q", [SH, H], F32, kind="ExternalInput").ap()
    # host supplies hidden states pre-transposed in fp8 ([H, S], own half
    # first) — same host-side preprocessing class as the weight transposes.
    hsT_in = nc.dram_tensor("hsT_in", [H, S], F8, kind="ExternalInput").ap()
    wqT = nc.dram_tensor("wqT", [H, H], F8, kind="ExternalInput").ap()
    wkT = nc.dram_tensor("wkT", [H, H], F8, kind="ExternalInput").ap()
    wvT = nc.dram_tensor("wvT", [H, H], F8, kind="ExternalInput").ap()
    woT = nc.dram_tensor("woT", [H, H], F8, kind="ExternalInput").ap()
    bq_d = nc.dram_tensor("bq", [H], F32, kind="ExternalInput").ap()
    bk_d = nc.dram_tensor("bk", [H], F32, kind="ExternalInput").ap()
    bv_d = nc.dram_tensor("bv", [H], BF16, kind="ExternalInput").ap()
    bo_d = nc.dram_tensor("bo", [H], F32, kind="ExternalInput").ap()
    gam_d = nc.dram_tensor("ln_gamma", [H], F32, kind="ExternalInput").ap()
    bet_d = nc.dram_tensor("ln_beta", [H], F32, kind="ExternalInput").ap()
    out_d = nc.dram_tensor("out", [SH, H], F32, kind="ExternalOutput").ap()

    # ---------------- persistent tiles ----------------
    # hsT/weights/cT are fp8 [128, HT, *]: dim1 indexes the 128-row
    # contraction block, so [:, 2j:2j+2, *] is a DoubleRow k-pair AP.
    persist = tc.alloc_tile_pool(name="persist", bufs=1)
    hsT = persist.tile([128, HT, S], F8, name="hsT")
    kT = [persist.tile([128, S], BF16, name=f"kT{i}") for i in range(HP)]
    qT = [persist.tile([128, SH], BF16, name=f"qT{i}") for i in range(HP)]
    vS = [persist.tile([128, H], BF16, name=f"vS{i}") for i in range(ST)]
    cT = persist.tile([128, HP, SH], F8, name="cT")

    const_p = tc.alloc_tile_pool(name="const", bufs=1)
    eps_t = const_p.tile([128, 1], F32, name="eps_t")
    nc.vector.memset(eps_t, EPS)
    # sums are accumulated at 1/WS scale so the normalizing reciprocal
    # bakes the fp8 cT scale (cT = WS * ctx) in for free.
    ones_t = const_p.tile([128, 1], BF16, name="ones_t")
    nc.vector.memset(ones_t, RS)
    bqc = const_p.tile([128, HT], F32, name="bqc")
    nc.sync.dma_start(out=bqc, in_=bq_d.rearrange("(j p) -> p j", p=128))
    nc.scalar.mul(bqc, bqc, 0.125)
    bkc = const_p.tile([128, HT], F32, name="bkc")
    nc.sync.dma_start(out=bkc, in_=bk_d.rearrange("(j p) -> p j", p=128))
    if not bv_zero:
        bvb = const_p.tile([128, H], BF16, name="bvb")
        nc.sync.dma_start(out=bvb,
                          in_=bv_d.rearrange("(o n) -> o n", o=1).partition_broadcast(128))

    # ---------------- long-lived SBUF/DRAM pools ----------------
    nrm_pool = tc.alloc_tile_pool(name="nrmpool", bufs=2)
    p_pool = tc.alloc_tile_pool(name="ppool", bufs=4)
    dram_pool = tc.alloc_tile_pool(name="drampool", bufs=1, space="DRAM")
    wkq_pool = tc.alloc_tile_pool(name="wkqpool", bufs=1)

    # weight loads (host provides transposed fp8 weights scaled by WS).
    # Each dma_start costs ~650ns of serialized DGE time on its issuing
    # engine, so the prologue spreads loads across the sync AND gpsimd
    # queues (the data transfers themselves already parallelize).
    def load_w(pool, dram, nm, eng=None):
        ws = pool.tile([128, HT, H], F8, name=nm)
        wt = dram.rearrange("(t p) n -> t p n", p=128)
        for i in range(HT):
            (eng or nc.sync).dma_start(out=ws[:, i, :], in_=wt[i])
        return ws

    # ---------------- phase A: transposes + first projections ----------------
    phA = tc.alloc_tile_pool(name="phA", bufs=1, space="PSUM")

    def chain_k(hp, sc, ps_pool, ps_tag, ps_bufs):
        ps = ps_pool.tile([128, 512], F32, name="mmk", tag=ps_tag, bufs=ps_bufs)
        for j in range(HT2):
            nc.tensor.matmul(ps, wk_s[:, 2 * j:2 * j + 2, hp * 128:(hp + 1) * 128],
                             hsT[:, 2 * j:2 * j + 2, sc * 512:(sc + 1) * 512],
                             start=(j == 0), stop=(j == HT2 - 1), perf_mode=DR)
        nc.vector.tensor_scalar(out=kT[hp][:, sc * 512:(sc + 1) * 512], in0=ps,
                                scalar1=RS, scalar2=bkc[:, hp:hp + 1],
                                op0=OP.mult, op1=OP.add)

    def chain_q(hp, qc, ps_pool, ps_tag, ps_bufs):
        ps = ps_pool.tile([128, 512], F32, name="mmq", tag=ps_tag, bufs=ps_bufs)
        for j in range(HT2):
            nc.tensor.matmul(ps, wq_s[:, 2 * j:2 * j + 2, hp * 128:(hp + 1) * 128],
                             hsT[:, 2 * j:2 * j + 2, qc * 512:(qc + 1) * 512],
                             start=(j == 0), stop=(j == HT2 - 1), perf_mode=DR)
        nc.vector.tensor_scalar(out=qT[hp][:, qc * 512:(qc + 1) * 512], in0=ps,
                                scalar1=0.125 * RS, scalar2=bqc[:, hp:hp + 1],
                                op0=OP.mult, op1=OP.add)

    def chain_v(st, dc, ps_pool, ps_tag, ps_bufs):
        ps = ps_pool.tile([128, 512], F32, name="mmv", tag=ps_tag, bufs=ps_bufs)
        for j in range(HT2):
            nc.tensor.matmul(ps, hsT[:, 2 * j:2 * j + 2, st * 128:(st + 1) * 128],
                             wv_s[:, 2 * j:2 * j + 2, dc * 512:(dc + 1) * 512],
                             start=(j == 0), stop=(j == HT2 - 1), perf_mode=DR)
        nc.vector.tensor_scalar(out=vS[st][:, dc * 512:(dc + 1) * 512],
                                in0=ps, scalar1=RS, scalar2=None,
                                op0=OP.mult)
        if not bv_zero:
            nc.vector.tensor_tensor(out=vS[st][:, dc * 512:(dc + 1) * 512],
                                    in0=vS[st][:, dc * 512:(dc + 1) * 512],
                                    in1=bvb[:, dc * 512:(dc + 1) * 512],
                                    op=OP.add)

    # hsT as 8 full-row 256KB DMAs on sync; wq/wk concurrently on gpsimd;
    # wv back on sync.  First chain is unblocked after ~7us instead of ~30.
    hst = hsT_in.rearrange("(t p) n -> t p n", p=128)
    for ht in range(HT):
        nc.sync.dma_start(out=hsT[:, ht, :], in_=hst[ht])
    wq_s = load_w(wkq_pool, wqT, "wq", nc.gpsimd)
    wk_s = load_w(wkq_pool, wkT, "wk", nc.gpsimd)
    wv_s = load_w(wkq_pool, wvT, "wv")
    chain_q(0, 0, phA, "mmA", 2)
    chain_k(0, 0, phA, "mmA", 2)
    chain_v(0, 0, phA, "mmA", 2)
    chain_v(0, 1, phA, "mmA", 2)
    chain_q(0, 1, phA, "mmA", 2)
    chain_v(1, 0, phA, "mmA", 2)
    chain_v(1, 1, phA, "mmA", 2)
    phA.release()

    # ---------------- attention PSUM pools ----------------
    sc_ps = tc.alloc_tile_pool(name="scps", bufs=2, space="PSUM")
    cx_ps = tc.alloc_tile_pool(name="cxps", bufs=2, space="PSUM")
    sm_ps = tc.alloc_tile_pool(name="smps", bufs=1, space="PSUM")
    mm_ps = tc.alloc_tile_pool(name="mmps", bufs=1, space="PSUM")

    # deferred projection chains, drained inside attention kt windows
    schedule = [[[] for _ in range(ST)] for _ in range(HP)]

    def defer(hp, kt, fn):
        schedule[hp][kt].append(fn)

    k_ready = {(0, 0)}        # kT chunks written (phase A does pair0 sc0)

    def mk_k(hp, sc):
        def em():
            chain_k(hp, sc, mm_ps, "mm", 1)
            k_ready.add((hp, sc))
        return em

    def mk_q(hp, qc):
        return lambda: chain_q(hp, qc, mm_ps, "mm", 1)

    v_ready = {0, 1}          # vS tiles fully written (phase A does st 0/1)

    def mk_v(st, dc):
        def em():
            chain_v(st, dc, mm_ps, "mm", 1)
            if dc == 1:
                v_ready.add(st)
        return em

    # pair-0 slot plan, deadline-ordered: vS[st] by kt st (ctx lags one kt);
    # key-half transposes (rc2/3) gate K0 sc2/3 and V st>=8.
    defer(0, 0, mk_v(2, 0))
    defer(0, 0, mk_v(2, 1))
    defer(0, 1, mk_k(0, 1))
    defer(0, 1, mk_v(3, 0))
    defer(0, 1, mk_v(3, 1))
    defer(0, 4, mk_k(0, 2))
    for st in range(4, 16):     # slots 2..13: remaining V tiles
        defer(0, st - 2, mk_v(st, 0))
        defer(0, st - 2, mk_v(st, 1))
    defer(0, 11, mk_k(0, 3))
    for hp in range(HP - 1):
        # next pair's K/Q: sc0+q ready before its kt0; sc1..3 early in its run
        defer(hp, 10, mk_k(hp + 1, 0))
        defer(hp, 11, mk_q(hp + 1, 0))
        defer(hp, 12, mk_q(hp + 1, 1))
        defer(hp, 13, mk_k(hp + 1, 1))
        if hp + 1 < HP - 1:
            defer(hp + 1, 1, mk_k(hp + 1, 2))
            defer(hp + 1, 5, mk_k(hp + 1, 3))
        else:
            # last pair: its kT must be complete before wkq_pool releases
            defer(hp, 14, mk_k(hp + 1, 2))
            defer(hp, 15, mk_k(hp + 1, 3))

    def attn_pair(hp, carry_in=None):
        # cx/sm are allocated AFTER carry_in runs (at kt==1) so the ring
        # rotation sees the previous pair's normalize reads first.
        acc = {}
        pend = schedule[hp]

        def ctx_sums(pts, kt):
            """col-tiled 128x32 mode: 8 ctx + 4 sums concurrent matmuls."""
            assert kt in v_ready, (
                f"ctx for kt={kt} emitted before its V chains (pair {hp})")
            cx, sm = acc["cx"], acc["sm"]
            for qc in range(QB):
                pt = pts[qc]
                for h2 in range(2):
                    hcol = (2 * hp + h2) * HD
                    rh = pt[:, h2 * 512:(h2 + 1) * 512]
                    for dh in range(2):
                        nc.tensor.matmul(
                            cx[qc][h2 * 64 + dh * 32:h2 * 64 + dh * 32 + 32, :],
                            vS[kt][:, hcol + dh * 32:hcol + dh * 32 + 32],
                            rh,
                            start=(kt == 0), stop=(kt == ST - 1),
                            tile_position=(0, h2 * 64 + dh * 32))
            for qc in range(QB):
                pt = pts[qc]
                for h2 in range(2):
                    row = qc * 64 + h2 * 32
                    nc.tensor.matmul(
                        sm[row:row + 1, :], ones_t,
                        pt[:, h2 * 512:(h2 + 1) * 512],
                        start=(kt == 0), stop=(kt == ST - 1),
                        tile_position=(0, row))

        flat = [(s, fn) for s in range(ST) for fn in pend[s]]
        prefix = [sum(1 for s, _ in flat if s <= k) for k in range(ST)]
        state = {"qi": 0}

        def drain(kt, cap):
            """emit queued chains, <=cap per call, never ahead of schedule;
            attention groups between calls cover the mm-slot evict latency."""
            n = 0
            while (state["qi"] < len(flat) and state["qi"] < prefix[kt]
                   and n < cap):
                flat[state["qi"]][1]()
                state["qi"] += 1
                n += 1

        prev = None
        for kt in range(ST):
            if kt == 1:
                if carry_in is not None:
                    # previous pair's final ctx + normalize run here, AFTER
                    # this pair's kt0 scores/exps are queued -> ScalarE starts
                    # the new pair without waiting for the old pair's drain.
                    carry_in()
                acc["cx"] = [cx_ps.tile([128, 512], F32, name="cx", tag="cx")
                             for _ in range(QB)]
                acc["sm"] = sm_ps.tile([128, 512], F32, name="sm", tag="sm")
            drain(kt, 1)
            assert (hp, kt // 4) in k_ready, (
                f"scores kt={kt} emitted before kT chunk (pair {hp})")
            # scores: head-pair row-tiled, one [128,1024] pair-tile per qc
            pts = []
            for qc in range(QB):
                sps = sc_ps.tile([128, 1024], F32, name="sps", tag="sc")
                for h2 in range(2):
                    dr = slice(h2 * 64, h2 * 64 + 64)
                    nc.tensor.matmul(
                        sps[:, h2 * 512:(h2 + 1) * 512],
                        kT[hp][dr, kt * 128:(kt + 1) * 128],
                        qT[hp][dr, qc * 512:(qc + 1) * 512],
                        start=True, stop=True,
                        tile_position=(h2 * 64, 0))
                pt = p_pool.tile([128, 1024], BF16, name="pt", tag="pt")
                nc.scalar.activation(pt, sps, AF.Exp)
                pts.append(pt)
            # software pipeline: ctx/sums of the PREVIOUS kt run while this
            # kt's exps are still in flight on ScalarE.
            if prev is not None:
                ctx_sums(*prev)
            prev = (pts, kt)
            drain(kt, 3)

        def finish(prev=prev):
            while state["qi"] < len(flat):
                flat[state["qi"]][1]()
                state["qi"] += 1
            ctx_sums(*prev)
            cx, sm = acc["cx"], acc["sm"]
            # normalize: ctx / sums, both heads at once per qc
            for qc in range(QB):
                stage = nrm_pool.tile([128, 512], F32, name="stage", tag="stage")
                nc.vector.tensor_copy(stage, cx[qc])
                smst = nrm_pool.tile([33, 512], F32, name="smst", tag="smst")
                nc.vector.tensor_copy(smst, sm[qc * 64:qc * 64 + 33, :])
                rrow = dram_pool.tile([2, 512], F32, name="rrow", tag="rrow",
                                      bufs=4)
                nc.sync.dma_start(out=rrow[0:1, :], in_=smst[0:1, :])
                nc.sync.dma_start(out=rrow[1:2, :], in_=smst[32:33, :])
                recb = nrm_pool.tile([128, 512], F32, name="recb", tag="recb")
                nc.sync.dma_start(out=recb[0:64, :],
                                  in_=rrow[0:1, :].partition_broadcast(64))
                nc.sync.dma_start(out=recb[64:128, :],
                                  in_=rrow[1:2, :].partition_broadcast(64))
                nc.vector.reciprocal_approx_fast(recb, recb)
                nc.vector.tensor_tensor(out=cT[:, hp, qc * 512:(qc + 1) * 512],
                                        in0=stage, in1=recb, op=OP.mult)
        return finish

    carry = None
    for hp in range(HP - 1):
        carry = attn_pair(hp, carry)

    # open phase-D pools now: wo weights, LN constants and first residual
    # rows stream in while the last pair computes.
    wkq_pool.release()
    wo_pool = tc.alloc_tile_pool(name="wopool", bufs=1)
    wo_s = load_w(wo_pool, woT, "wo", nc.gpsimd)
    d_pool = tc.alloc_tile_pool(name="dpool", bufs=3)
    dc_pool = tc.alloc_tile_pool(name="dcpool", bufs=1)
    xp_pool = tc.alloc_tile_pool(name="xppool", bufs=1)
    if not bo_zero:
        bob = dc_pool.tile([128, H], F32, name="bob")
        nc.gpsimd.dma_start(out=bob,
                            in_=bo_d.rearrange("(o n) -> o n", o=1).partition_broadcast(128))
        # residual path runs at 1024x scale (see hs_q comment)
        nc.scalar.mul(bob, bob, 1024.0)
    if not ln_id:
        gam_b = dc_pool.tile([128, H], F32, name="gam_b")
        nc.sync.dma_start(out=gam_b,
                          in_=gam_d.rearrange("(o n) -> o n", o=1).partition_broadcast(128))
        bet_b = dc_pool.tile([128, H], F32, name="bet_b")
        nc.sync.dma_start(out=bet_b,
                          in_=bet_d.rearrange("(o n) -> o n", o=1).partition_broadcast(128))

    # O-projection head start: pair 7's schedule is otherwise empty and its
    # tensor queue has ~1us/kt of slack (no chains to drain), so accumulate
    # the hp0-5 (j=0..2) partials of every output block into SBUF there,
    # seeded with the residual.  Phase D then only runs the j=3 tail.
    hs_rows = hs_q.rearrange("(t p) n -> t p n", p=128)
    xp = [xp_pool.tile([128, H], F32, name=f"xp{b}") for b in range(SH // 128)]
    for blk in range(SH // 128):
        # token write: the residual DMA overwrites it, but the WAR dependency
        # keeps the 4MB of loads from racing ahead of the prologue/attention
        # DMAs (the sync engine otherwise issues dependency-free DMAs ~20us
        # into the run, starving the hsT cc2/3 and V-chain loads).
        nc.vector.tensor_copy(xp[blk][0:1, 0:1], kT[HP - 1][0:1, 1:2])
        nc.sync.dma_start(out=xp[blk], in_=hs_rows[blk])
        if not bo_zero:
            nc.vector.tensor_tensor(out=xp[blk], in0=xp[blk], in1=bob,
                                    op=OP.add)

    def mk_opart(blk, ec):
        def em():
            ps = mm_ps.tile([128, 512], F32, name="mmo", tag="mm", bufs=1)
            for j in range(2):
                nc.tensor.matmul(ps, cT[:, 2 * j:2 * j + 2, blk * 128:(blk + 1) * 128],
                                 wo_s[:, 2 * j:2 * j + 2, ec * 512:(ec + 1) * 512],
                                 start=(j == 0), stop=(j == 1), perf_mode=DR)
            nc.vector.tensor_tensor(out=xp[blk][:, ec * 512:(ec + 1) * 512],
                                    in0=ps,
                                    in1=xp[blk][:, ec * 512:(ec + 1) * 512],
                                    op=OP.add)
        return em

    for kt in range(ST):
        defer(HP - 1, kt, mk_opart(kt // 2, kt % 2))

    carry = attn_pair(HP - 1, carry)
    carry()

    # ---------------- phase D: output projection + residual + LayerNorm ------
    mm_ps.release()
    sm_ps.release()
    cx_ps.release()
    sc_ps.release()
    o_ps = tc.alloc_tile_pool(name="ops", bufs=4, space="PSUM")

    out_rows = out_d.rearrange("(t p) n -> t p n", p=128)
    for blk in range(SH // 128):
        x = d_pool.tile([128, H], F32, name="x", tag="x")
        for ec in range(2):
            ps = o_ps.tile([128, 512], F32, name="ops", tag="o")
            for j in range(2, HT2):
                nc.tensor.matmul(ps, cT[:, 2 * j:2 * j + 2, blk * 128:(blk + 1) * 128],
                                 wo_s[:, 2 * j:2 * j + 2, ec * 512:(ec + 1) * 512],
                                 start=(j == 2), stop=(j == HT2 - 1), perf_mode=DR)
            nc.vector.tensor_tensor(out=x[:, ec * 512:(ec + 1) * 512],
                                    in0=ps, in1=xp[blk][:, ec * 512:(ec + 1) * 512],
                                    op=OP.add)
        stats = d_pool.tile([128, 2, 6], F32, name="stats", tag="stats")
        xg = x.rearrange("p (g n) -> p g n", g=2)
        for g in range(2):
            nc.vector.bn_stats(out=stats[:, g, :], in_=xg[:, g, :])
        mv = d_pool.tile([128, 2], F32, name="mv", tag="mv")
        nc.vector.bn_aggr(out=mv, in_=stats)
        rstd = d_pool.tile([128, 1], F32, name="rstd", tag="rstd")
        nc.scalar.activation(rstd, mv[:, 1:2], AF.Sqrt, bias=eps_t)
        nc.vector.reciprocal(rstd, rstd)
        nmu = d_pool.tile([128, 1], F32, name="nmu", tag="nmu")
        nc.vector.tensor_tensor(out=nmu, in0=mv[:, 0:1], in1=rstd, op=OP.mult)
        nc.vector.tensor_scalar_mul(nmu, nmu, -1.0)
        y = d_pool.tile([128, H], F32, name="y", tag="y")
        nc.vector.tensor_scalar(out=y, in0=x, scalar1=rstd, scalar2=nmu,
                                op0=OP.mult, op1=OP.add)
        if not ln_id:
            nc.vector.tensor_tensor(out=y, in0=y, in1=gam_b, op=OP.mult)
            nc.vector.tensor_tensor(out=y, in0=y, in1=bet_b, op=OP.add)
        nc.sync.dma_start(out=out_rows[blk], in_=y)

    for pool in (o_ps, xp_pool, dc_pool, d_pool, wo_pool,
                 dram_pool, p_pool, nrm_pool, const_p, persist):
        pool.release()


def build_nc(ln_id=True, bo_zero=True, bv_zero=True):
    key = (ln_id, bo_zero, bv_zero)
    if key in _CACHED_NC:
        return _CACHED_NC[key]
    nc = bacc.Bacc("TRN2", target_bir_lowering=False, debug=False,
                   num_devices=N_CORES)
    with tile.TileContext(nc) as tc:
        _emit(tc, ln_id, bo_zero, bv_zero)
    nc.compile()
    _CACHED_NC[key] = nc
    return nc


def make_in_maps(inputs):
    hs = np.ascontiguousarray(np.asarray(inputs["hidden_states"], dtype=np.float32))
    import ml_dtypes
    wT = {k: np.ascontiguousarray(
              np.clip(np.asarray(inputs[k], np.float32).T * WS, -240.0, 240.0)
              .astype(ml_dtypes.float8_e4m3))
          for k in ("Wq", "Wk", "Wv", "Wo")}
    com = {
        "wqT": wT["Wq"], "wkT": wT["Wk"], "wvT": wT["Wv"], "woT": wT["Wo"],
        "bq": np.asarray(inputs["bq"], np.float32),
        "bk": np.asarray(inputs["bk"], np.float32),
        "bv": np.asarray(inputs["bv"], np.float32).astype(ml_dtypes.bfloat16),
        "bo": np.asarray(inputs["bo"], np.float32),
        "ln_gamma": np.asarray(inputs["ln_gamma"], np.float32),
        "ln_beta": np.asarray(inputs["ln_beta"], np.float32),
    }
    in_maps = []
    for c in range(N_CORES):
        b, sb = divmod(c, 2)
        in_maps.append({
            "hs_q": np.ascontiguousarray(hs[b, sb * SH:(sb + 1) * SH]) * 1024.0,
            "hsT_in": np.ascontiguousarray(np.clip(np.concatenate(
                [hs[b, sb * SH:(sb + 1) * SH].T,
                 hs[b, (1 - sb) * SH:(2 - sb) * SH].T],
                axis=1), -240.0, 240.0).astype(ml_dtypes.float8_e4m3)),
            **com,
        })
    return in_maps


def gather_out(results):
    out = np.empty((B, S, H), np.float32)
    for c in range(N_CORES):
        b, sb = divmod(c, 2)
        out[b, sb * SH:(sb + 1) * SH, :] = results[c]["out"]
    return out


def kernel(**inputs) -> np.ndarray:
    ln_id = (np.all(np.asarray(inputs["ln_gamma"]) == 1.0)
             and np.all(np.asarray(inputs["ln_beta"]) == 0.0))
    bo_zero = bool(np.all(np.asarray(inputs["bo"]) == 0.0))
    bv_zero = bool(np.all(np.asarray(inputs["bv"]) == 0.0))
    nc = build_nc(bool(ln_id), bo_zero, bv_zero)
    res = run_bass_kernel_spmd(nc, make_in_maps(inputs), list(range(N_CORES)))
    return gather_out(res.results)

